# revision 1
# baseline (speedup 1.0000x reference)
"""NoisyRouter Trainium2 kernel.

Computes, for x:[B,S,D] f32, gate_w/noise_w:[E,D], gate_b/noise_b:[E],
gauss:[B,S,E]:
    logits       = x @ gate_w.T + gate_b
    noise_logits = x @ noise_w.T + noise_b
    noisy        = logits + gauss * softplus(noise_logits)
    top_vals, top_idx = top_k(noisy, 8)
    sparse_logits = softmax(scatter(-inf, top_idx, top_vals))
Returns (sparse_logits:[B,S,E] f32, top_idx:[B,S,8] int32).

Distribution: pure data-parallel over tokens — each of the 8 NeuronCores
gets B*S/8 = 2048 tokens; the small router weights are replicated.

Per-core pipeline (all fp32 data, fp16 hi/lo "3x" matmul for speed):
  x rows stream in natural [token, D] layout -> PE transposes 128x128
  tiles into PSUM (x^T) -> ACT copies PSUM->SBUF casting to fp16 (hi)
  while DVE computes lo = fp32(x^T) - hi (fp16). The dual-projection
  matmul runs expert-major with the gate|noise weight pair packed on
  128 partitions: logits^T[pair, tok] accumulates w_hi.T@x_hi +
  w_hi.T@x_lo + w_lo.T@x_hi (fp16 pair arithmetic ~= fp32 precision,
  1 PE cycle/row instead of 4 for fp32). Biases enter as rank-1 K=1
  matmuls. logits^T transposes back to token-major, then per 128-token
  tile: softplus(noise) = Ln(Exp(noise)+1) on ACT, noisy on DVE, DVE
  max8/max_index give the top-8 (sorted) values+indices, and the masked
  softmax uses a fused compare-multiply-accumulate against the 8th max.
"""
import sys

sys.path.insert(0, "/opt/trn_rl_repo")

import numpy as np

import concourse.bass as bass
import concourse.tile as tile
from concourse import mybir
from concourse.vector_clock import ScopedClock

F32 = mybir.dt.float32
F16 = mybir.dt.float16
U32 = mybir.dt.uint32
AF = mybir.ActivationFunctionType
ALU = mybir.AluOpType

B, S, DIM, E, TOP_K = 4, 4096, 4096, 64, 8
N_CORES = 8
N_TOK = B * S
T_CORE = N_TOK // N_CORES          # 2048 tokens per core
PAIR = 2 * E                       # gate|noise packed: 128
CH = DIM // 128                    # 32 contraction chunks
BLOCK = 512                        # tokens per compute block
NB = T_CORE // BLOCK               # blocks per core
TPB = BLOCK // 128                 # 128-token tiles per block: 4


class _TileContextSplitWaits(tile.TileContext):
    """The walrus codegen used here accepts only ONE sync-wait command per
    instruction; Tile can emit several. Split the extras onto same-engine
    NOPs placed immediately before the instruction."""

    def _drain_and_barrier(self, tick_clock, wait_clock):
        nc = self.nc
        drain_inst = nc.sync.drain()
        wait_clock.add_sem_waits(
            drain_inst.ins, ScopedClock({None: tick_clock.global_clock})
        )
        nc.all_engine_barrier()
        assert self.sems is not None
        popped = nc._tile_sem_poison_stack.pop()
        assert popped is self._sem_poison
        nc.clear_and_free_semaphores(list(self.sems.allocated().values()))
        nc.all_engine_barrier()

    def schedule_and_allocate(self):
        ret = super().schedule_and_allocate()
        nc = self.nc
        for bb in nc.bb_map.values():
            insts = bb.bb.instructions
            i = 0
            while i < len(insts):
                inst = insts[i]
                si = inst.sync_info
                if si is not None and si.on_wait and len(si.on_wait) > 1:
                    waits = list(si.on_wait)
                    si.on_wait = [waits[-1]]
                    for k, w in enumerate(waits[:-1]):
                        nop = mybir.InstNoOp(
                            name=f"{inst.name}-w{k}",
                            engine=inst.engine,
                            ins=[],
                            outs=[],
                            sync_info=mybir.SyncInfo(on_wait=[w], on_update=[]),
                        )
                        nc.register_instruction(nop, overwrite=True)
                        insts.insert(i, nop)
                        i += 1
                i += 1
        return ret


def _build_program():
    nc = bass.Bass(
        "TRN2", target_bir_lowering=False, debug=False, num_devices=N_CORES
    )
    x_d = nc.dram_tensor("x", [T_CORE, DIM], F32, kind="ExternalInput").ap()
    # weights pre-arranged on host to the SBUF-native layout
    # wt_sb[p, c*128 + e] = w_pair[e, c*128 + p]
    wt_d = nc.dram_tensor("wt", [128, CH * PAIR], F32, kind="ExternalInput").ap()
    bh_d = nc.dram_tensor("bh", [1, PAIR], F16, kind="ExternalInput").ap()
    bl_d = nc.dram_tensor("bl", [1, PAIR], F16, kind="ExternalInput").ap()
    # gauss pre-arranged: g_sb[p, ((b*TPB+tt)*E + e)] = gauss[b*BLOCK+tt*128+p, e]
    g_d = nc.dram_tensor("g", [128, NB * TPB * E], F32, kind="ExternalInput").ap()
    eye_d = nc.dram_tensor("eye", [128, 128], F32, kind="ExternalInput").ap()
    # outputs in the same partition-major layout; host undoes it
    pr_d = nc.dram_tensor("probs", [128, NB * TPB * E], F32, kind="ExternalOutput").ap()
    ix_d = nc.dram_tensor(
        "idx", [128, NB * TPB * TOP_K], U32, kind="ExternalOutput"
    ).ap()

    with _TileContextSplitWaits(nc) as tc:
        _emit(nc, tc, x_d, wt_d, bh_d, bl_d, g_d, eye_d, pr_d, ix_d)
    return nc


def _emit(nc, tc, x_d, wt_d, bh_d, bl_d, g_d, eye_d, pr_d, ix_d):
    from contextlib import ExitStack

    ctx = ExitStack()
    with ctx:
        const = ctx.enter_context(tc.tile_pool(name="const", bufs=1))
        xpool = ctx.enter_context(tc.tile_pool(name="xpool", bufs=6))
        stage = ctx.enter_context(tc.tile_pool(name="stage", bufs=2))
        rout = ctx.enter_context(tc.tile_pool(name="rout", bufs=3))
        xtps = ctx.enter_context(tc.tile_pool(name="xtps", bufs=2, space="PSUM"))
        lps = ctx.enter_context(tc.tile_pool(name="lps", bufs=2, space="PSUM"))
        nps = ctx.enter_context(tc.tile_pool(name="nps", bufs=2, space="PSUM"))

        # ---- setup: constants, weights ----
        eye = const.tile([128, 128], F32)
        nc.sync.dma_start(eye[:], eye_d[:])
        w_raw = const.tile([128, CH * PAIR], F32)
        nc.sync.dma_start(w_raw[:], wt_d[:])
        gs = const.tile([128, NB * TPB * E], F32)
        nc.sync.dma_start(gs[:], g_d[:])
        bh_s = const.tile([1, PAIR], F16)
        nc.sync.dma_start(bh_s[:], bh_d[:])
        bl_s = const.tile([1, PAIR], F16)
        nc.sync.dma_start(bl_s[:], bl_d[:])
        ones = const.tile([1, BLOCK], F16)
        nc.vector.memset(ones[:], 1.0)
        wh = const.tile([128, CH * PAIR], F16)
        nc.scalar.copy(wh[:], w_raw[:])
        wl = const.tile([128, CH * PAIR], F16)
        nc.vector.scalar_tensor_tensor(
            wl[:], w_raw[:], 0.0, wh[:], op0=ALU.add, op1=ALU.subtract
        )

        for b in range(NB):
            # ---- stream x block in (token-major) ----
            xs = []
            for tt in range(TPB):
                xt_in = xpool.tile([128, DIM], F32, name=f"x_{b}_{tt}", tag="x")
                nc.sync.dma_start(
                    xt_in[:], x_d[b * BLOCK + tt * 128 : b * BLOCK + (tt + 1) * 128, :]
                )
                xs.append(xt_in)

            lp = lps.tile([128, BLOCK], F32, name=f"lp{b}", tag="lp")
            # biases as rank-1 K=1 matmuls open the accumulation group
            nc.tensor.matmul(lp[:], bh_s[0:1, :], ones[0:1, :], start=True, stop=False)
            nc.tensor.matmul(lp[:], bl_s[0:1, :], ones[0:1, :], start=False, stop=False)

            # ---- transpose x, split fp16 hi/lo, dual-projection matmul ----
            for g2 in range(CH // 2):
                xt = xtps.tile([128, 2 * BLOCK], F32, name=f"xt{b}_{g2}", tag="xt")
                for j in range(2):
                    c = g2 * 2 + j
                    for tt in range(TPB):
                        nc.tensor.transpose(
                            xt[:, j * BLOCK + tt * 128 : j * BLOCK + (tt + 1) * 128],
                            xs[tt][:, c * 128 : (c + 1) * 128],
                            eye[:],
                        )
                xh = stage.tile([128, 2 * BLOCK], F16, name=f"xh{b}_{g2}", tag="xh")
                nc.scalar.copy(xh[:], xt[:])
                xl = stage.tile([128, 2 * BLOCK], F16, name=f"xl{b}_{g2}", tag="xl")
                nc.vector.scalar_tensor_tensor(
                    xl[:], xt[:], 0.0, xh[:], op0=ALU.add, op1=ALU.subtract
                )
                for j in range(2):
                    c = g2 * 2 + j
                    wsl = slice(c * PAIR, (c + 1) * PAIR)
                    xsl = slice(j * BLOCK, (j + 1) * BLOCK)
                    last = c == CH - 1
                    nc.tensor.matmul(
                        lp[:], wh[:, wsl], xh[:, xsl], start=False, stop=False
                    )
                    nc.tensor.matmul(
                        lp[:], wh[:, wsl], xl[:, xsl], start=False, stop=False
                    )
                    nc.tensor.matmul(
                        lp[:], wl[:, wsl], xh[:, xsl], start=False, stop=last
                    )

            # ---- back to token-major ----
            lg = stage.tile([128, BLOCK], F32, name=f"lg{b}", tag="lg")
            nc.scalar.copy(lg[:], lp[:])
            npb = nps.tile([128, BLOCK], F32, name=f"np{b}", tag="np")
            for tt in range(TPB):
                nc.tensor.transpose(
                    npb[:, tt * 128 : (tt + 1) * 128],
                    lg[:, tt * 128 : (tt + 1) * 128],
                    eye[:],
                )

            # ---- router math per 128-token tile ----
            prs = rout.tile([128, TPB * E], F32, name=f"prs{b}", tag="prs")
            ixs = rout.tile([128, TPB * TOP_K], U32, name=f"ixs{b}", tag="ixs")
            for tt in range(TPB):
                gate = npb[:, tt * 128 : tt * 128 + E]
                noise = npb[:, tt * 128 + E : tt * 128 + PAIR]
                en = rout.tile([128, E], F32, name=f"en{b}_{tt}", tag="en")
                nc.scalar.activation(en[:], noise, AF.Exp)
                sp = rout.tile([128, E], F32, name=f"sp{b}_{tt}", tag="sp")
                nc.scalar.activation(sp[:], en[:], AF.Ln, bias=1.0)
                gsl = gs[:, (b * TPB + tt) * E : (b * TPB + tt + 1) * E]
                pr = rout.tile([128, E], F32, name=f"pr{b}_{tt}", tag="pr")
                nc.vector.tensor_tensor(pr[:], gsl, sp[:], op=ALU.mult)
                nz = rout.tile([128, E], F32, name=f"nz{b}_{tt}", tag="nz")
                nc.vector.tensor_tensor(nz[:], gate, pr[:], op=ALU.add)
                mx = rout.tile([128, TOP_K], F32, name=f"mx{b}_{tt}", tag="mx")
                nc.vector.max(mx[:], nz[:])
                nc.vector.max_index(
                    ixs[:, tt * TOP_K : (tt + 1) * TOP_K], mx[:], nz[:]
                )
                ez = rout.tile([128, E], F32, name=f"ez{b}_{tt}", tag="ez")
                nc.scalar.activation(ez[:], nz[:], AF.Exp)
                me = rout.tile([128, E], F32, name=f"me{b}_{tt}", tag="me")
                ssum = rout.tile([128, 1], F32, name=f"ss{b}_{tt}", tag="ss")
                nc.vector.scalar_tensor_tensor(
                    me[:], nz[:], mx[:, TOP_K - 1 : TOP_K], ez[:],
                    op0=ALU.is_ge, op1=ALU.mult, accum_out=ssum[:],
                )
                rcp = rout.tile([128, 1], F32, name=f"rc{b}_{tt}", tag="rc")
                nc.vector.reciprocal(rcp[:], ssum[:])
                nc.vector.tensor_scalar(
                    prs[:, tt * E : (tt + 1) * E], me[:], rcp[:, 0:1], None,
                    op0=ALU.mult,
                )

            nc.sync.dma_start(
                pr_d[:, b * TPB * E : (b + 1) * TPB * E], prs[:]
            )
            nc.sync.dma_start(
                ix_d[:, b * TPB * TOP_K : (b + 1) * TPB * TOP_K], ixs[:]
            )


_NC_CACHE = None


def _get_program():
    global _NC_CACHE
    if _NC_CACHE is None:
        _NC_CACHE = _build_program()
    return _NC_CACHE


def kernel(x, gate_w, gate_b, noise_w, noise_b, gauss):
    from concourse.bass_utils import run_bass_kernel_spmd

    x = np.asarray(x, dtype=np.float32)
    gauss = np.asarray(gauss, dtype=np.float32)
    gate_w = np.asarray(gate_w, dtype=np.float32)
    noise_w = np.asarray(noise_w, dtype=np.float32)
    gate_b = np.asarray(gate_b, dtype=np.float32)
    noise_b = np.asarray(noise_b, dtype=np.float32)

    # host-side layout prep (all small / cheap except x, which is sliced only)
    w_pair = np.concatenate([gate_w, noise_w], axis=0)            # [128, D]
    # wt_sb[p, c*PAIR + e] = w_pair[e, c*128 + p]
    wt_sb = np.ascontiguousarray(
        w_pair.T.reshape(CH, 128, PAIR).transpose(1, 0, 2).reshape(128, CH * PAIR)
    )
    b_pair = np.concatenate([gate_b, noise_b]).astype(np.float32)
    bh = b_pair.astype(np.float16)
    bl = (b_pair - bh.astype(np.float32)).astype(np.float16)
    eye = np.eye(128, dtype=np.float32)

    x2 = x.reshape(N_TOK, DIM)
    g2 = gauss.reshape(N_TOK, E)

    in_maps = []
    for i in range(N_CORES):
        gsl = g2[i * T_CORE : (i + 1) * T_CORE]
        g_sb = np.ascontiguousarray(
            gsl.reshape(NB * TPB, 128, E).transpose(1, 0, 2).reshape(128, NB * TPB * E)
        )
        in_maps.append(
            {
                "x": x2[i * T_CORE : (i + 1) * T_CORE],
                "wt": wt_sb,
                "bh": bh.reshape(1, PAIR),
                "bl": bl.reshape(1, PAIR),
                "g": g_sb,
                "eye": eye,
            }
        )

    nc = _get_program()
    res = run_bass_kernel_spmd(nc, in_maps, core_ids=list(range(N_CORES)))

    probs = np.empty((N_TOK, E), dtype=np.float32)
    idx = np.empty((N_TOK, TOP_K), dtype=np.int32)
    for i in range(N_CORES):
        r = res.results[i]
        p = r["probs"].reshape(128, NB * TPB, E).transpose(1, 0, 2).reshape(T_CORE, E)
        probs[i * T_CORE : (i + 1) * T_CORE] = p
        ii = (
            r["idx"]
            .view(np.int32)
            .reshape(128, NB * TPB, TOP_K)
            .transpose(1, 0, 2)
            .reshape(T_CORE, TOP_K)
        )
        idx[i * T_CORE : (i + 1) * T_CORE] = ii
    return probs.reshape(B, S, E), idx.reshape(B, S, TOP_K)


# revision 2
# speedup vs baseline: 1.4028x; 1.4028x over previous
"""NoisyRouter Trainium2 kernel.

Computes, for x:[B,S,D] f32, gate_w/noise_w:[E,D], gate_b/noise_b:[E],
gauss:[B,S,E]:
    logits       = x @ gate_w.T + gate_b
    noise_logits = x @ noise_w.T + noise_b
    noisy        = logits + gauss * softplus(noise_logits)
    top_vals, top_idx = top_k(noisy, 8)
    sparse_logits = softmax(scatter(-inf, top_idx, top_vals))
Returns (sparse_logits:[B,S,E] f32, top_idx:[B,S,8] int32).

Distribution: pure data-parallel over tokens — each of the 8 NeuronCores
gets B*S/8 = 2048 tokens; the small router weights are replicated.

Per-core pipeline (fp32 data, fp16 hi/lo "3x" matmul for speed):
  x rows stream in natural [token, D] layout -> PE transposes 128x128
  tiles into PSUM (x^T) -> ACT copies PSUM->SBUF casting to fp16 (hi)
  while DVE computes lo = fp32(x^T) - hi (fp16). The dual-projection
  matmul runs expert-major with the gate|noise weight pair packed on
  128 partitions: logits^T[pair, tok] accumulates w_hi.T@x_hi +
  w_hi.T@x_lo + w_lo.T@x_hi (fp16 pair arithmetic ~= fp32 precision at
  1 PE cycle/row instead of 4 for fp32). Biases enter as rank-1 K=1
  matmuls. The PE instruction stream is software-pipelined in batches
  (transposes of batch k+1 between the matmuls of batches k and k+1) so
  matmul bursts stay long enough to hold the PE clock at full rate.
  logits^T transposes back to token-major, then per 128-token tile:
  softplus(noise) = Ln(Exp(noise)+1) on ACT, noisy on DVE, DVE
  max8/max_index give the top-8 (sorted) values+indices, and the masked
  softmax uses a fused compare-multiply-accumulate against the 8th max.
"""
import sys

sys.path.insert(0, "/opt/trn_rl_repo")

import numpy as np

import concourse.bass as bass
import concourse.tile as tile
from concourse import mybir
from concourse.vector_clock import ScopedClock

F32 = mybir.dt.float32
F16 = mybir.dt.float16
U32 = mybir.dt.uint32
AF = mybir.ActivationFunctionType
ALU = mybir.AluOpType

B, S, DIM, E, TOP_K = 4, 4096, 4096, 64, 8
N_CORES = 8
N_TOK = B * S
T_CORE = N_TOK // N_CORES          # 2048 tokens per core
PAIR = 2 * E                       # gate|noise packed: 128
CH = DIM // 128                    # 32 contraction chunks
BLOCK = 512                        # tokens per compute block
NB = T_CORE // BLOCK               # blocks per core
TPB = BLOCK // 128                 # 128-token tiles per block: 4
GRP = 2                            # contraction chunks per hi/lo staging group
NG = CH // GRP                     # 16 groups per block
BATCH = 3                          # staging groups per PE pipeline batch


class _TileContextSplitWaits(tile.TileContext):
    """The walrus codegen used here accepts only ONE sync-wait command per
    instruction; Tile can emit several. Split the extras onto same-engine
    NOPs placed immediately before the instruction."""

    def _drain_and_barrier(self, tick_clock, wait_clock):
        nc = self.nc
        drain_inst = nc.sync.drain()
        wait_clock.add_sem_waits(
            drain_inst.ins, ScopedClock({None: tick_clock.global_clock})
        )
        nc.all_engine_barrier()
        assert self.sems is not None
        popped = nc._tile_sem_poison_stack.pop()
        assert popped is self._sem_poison
        nc.clear_and_free_semaphores(list(self.sems.allocated().values()))
        nc.all_engine_barrier()

    def schedule_and_allocate(self):
        ret = super().schedule_and_allocate()
        nc = self.nc
        for bb in nc.bb_map.values():
            insts = bb.bb.instructions
            i = 0
            while i < len(insts):
                inst = insts[i]
                si = inst.sync_info
                if si is not None and si.on_wait and len(si.on_wait) > 1:
                    waits = list(si.on_wait)
                    si.on_wait = [waits[-1]]
                    for k, w in enumerate(waits[:-1]):
                        nop = mybir.InstNoOp(
                            name=f"{inst.name}-w{k}",
                            engine=inst.engine,
                            ins=[],
                            outs=[],
                            sync_info=mybir.SyncInfo(on_wait=[w], on_update=[]),
                        )
                        nc.register_instruction(nop, overwrite=True)
                        insts.insert(i, nop)
                        i += 1
                i += 1
        return ret


def _build_program():
    nc = bass.Bass(
        "TRN2", target_bir_lowering=False, debug=False, num_devices=N_CORES
    )
    x_d = nc.dram_tensor("x", [T_CORE, DIM], F32, kind="ExternalInput").ap()
    # weights pre-arranged on host to the SBUF-native fp16 hi/lo layout:
    # w?_sb[p, c*PAIR + e] = hi/lo(w_pair[e, c*128 + p])
    wh_d = nc.dram_tensor("wh", [128, CH * PAIR], F16, kind="ExternalInput").ap()
    wl_d = nc.dram_tensor("wl", [128, CH * PAIR], F16, kind="ExternalInput").ap()
    bh_d = nc.dram_tensor("bh", [1, PAIR], F16, kind="ExternalInput").ap()
    bl_d = nc.dram_tensor("bl", [1, PAIR], F16, kind="ExternalInput").ap()
    # gauss pre-arranged: g_sb[p, ((b*TPB+tt)*E + e)] = gauss[b*BLOCK+tt*128+p, e]
    g_d = nc.dram_tensor("g", [128, NB * TPB * E], F32, kind="ExternalInput").ap()
    eye_d = nc.dram_tensor("eye", [128, 128], F32, kind="ExternalInput").ap()
    # outputs in the same partition-major layout; host undoes it
    pr_d = nc.dram_tensor("probs", [128, NB * TPB * E], F32, kind="ExternalOutput").ap()
    ix_d = nc.dram_tensor(
        "idx", [128, NB * TPB * TOP_K], U32, kind="ExternalOutput"
    ).ap()

    with _TileContextSplitWaits(nc) as tc:
        _emit(nc, tc, x_d, wh_d, wl_d, bh_d, bl_d, g_d, eye_d, pr_d, ix_d)
    return nc


def _emit(nc, tc, x_d, wh_d, wl_d, bh_d, bl_d, g_d, eye_d, pr_d, ix_d):
    from contextlib import ExitStack

    ctx = ExitStack()
    with ctx:
        const = ctx.enter_context(tc.tile_pool(name="const", bufs=1))
        xpool = ctx.enter_context(tc.tile_pool(name="xpool", bufs=8))
        stage = ctx.enter_context(tc.tile_pool(name="stage", bufs=3))
        rout = ctx.enter_context(tc.tile_pool(name="rout", bufs=3))
        xtps = ctx.enter_context(tc.tile_pool(name="xtps", bufs=BATCH, space="PSUM"))
        lps = ctx.enter_context(tc.tile_pool(name="lps", bufs=1, space="PSUM"))
        nps = ctx.enter_context(tc.tile_pool(name="nps", bufs=1, space="PSUM"))

        # eye first (transposes need it), then block-0 x, then weights,
        # then gauss — DMA queue order controls when compute can start.
        eye = const.tile([128, 128], F32)
        nc.sync.dma_start(eye[:], eye_d[:])

        xs_all = []
        def load_block(b):
            xs = []
            for tt in range(TPB):
                xt_in = xpool.tile([128, DIM], F32, name=f"x_{b}_{tt}", tag="x")
                nc.sync.dma_start(
                    xt_in[:],
                    x_d[b * BLOCK + tt * 128 : b * BLOCK + (tt + 1) * 128, :],
                )
                xs.append(xt_in)
            return xs

        xs_all.append(load_block(0))

        wh = const.tile([128, CH * PAIR], F16)
        nc.sync.dma_start(wh[:], wh_d[:])
        wl = const.tile([128, CH * PAIR], F16)
        nc.sync.dma_start(wl[:], wl_d[:])
        bh_s = const.tile([1, PAIR], F16)
        nc.sync.dma_start(bh_s[:], bh_d[:])
        bl_s = const.tile([1, PAIR], F16)
        nc.sync.dma_start(bl_s[:], bl_d[:])
        ones = const.tile([1, BLOCK], F16)
        nc.vector.memset(ones[:], 1.0)
        gs = const.tile([128, NB * TPB * E], F32)
        nc.sync.dma_start(gs[:], g_d[:])

        for b in range(NB):
            if b + 1 < NB:
                xs_all.append(load_block(b + 1))
            xs = xs_all[b]

            lp = lps.tile([128, BLOCK], F32, name=f"lp{b}", tag="lp")
            nc.tensor.matmul(lp[:], bh_s[0:1, :], ones[0:1, :], start=True, stop=False)
            nc.tensor.matmul(lp[:], bl_s[0:1, :], ones[0:1, :], start=False, stop=False)

            # ---- software-pipelined PE stream over staging groups ----
            # batch k: transposes emitted before batch k-1's matmuls so the
            # PE alternates [T-batch][M-batch] with no bubbles and long
            # matmul bursts.
            def transpose_group(g2):
                xt = xtps.tile(
                    [128, GRP * BLOCK], F32, name=f"xt{b}_{g2}", tag="xt"
                )
                for j in range(GRP):
                    c = g2 * GRP + j
                    for tt in range(TPB):
                        nc.tensor.transpose(
                            xt[:, j * BLOCK + tt * 128 : j * BLOCK + (tt + 1) * 128],
                            xs[tt][:, c * 128 : (c + 1) * 128],
                            eye[:],
                        )
                xh = stage.tile([128, GRP * BLOCK], F16, name=f"xh{b}_{g2}", tag="xh")
                nc.scalar.copy(xh[:], xt[:])
                xl = stage.tile([128, GRP * BLOCK], F16, name=f"xl{b}_{g2}", tag="xl")
                nc.vector.scalar_tensor_tensor(
                    xl[:], xt[:], 0.0, xh[:], op0=ALU.add, op1=ALU.subtract
                )
                return xh, xl

            def matmul_group(g2, xh, xl):
                for j in range(GRP):
                    c = g2 * GRP + j
                    wsl = slice(c * PAIR, (c + 1) * PAIR)
                    xsl = slice(j * BLOCK, (j + 1) * BLOCK)
                    last = c == CH - 1
                    nc.tensor.matmul(
                        lp[:], wh[:, wsl], xh[:, xsl], start=False, stop=False
                    )
                    nc.tensor.matmul(
                        lp[:], wh[:, wsl], xl[:, xsl], start=False, stop=False
                    )
                    nc.tensor.matmul(
                        lp[:], wl[:, wsl], xh[:, xsl], start=False, stop=last
                    )

            pend = []  # [(g2, xh, xl)] transposed but not yet matmul'ed
            for k0 in range(0, NG, BATCH):
                batch = list(range(k0, min(k0 + BATCH, NG)))
                for g2 in batch:
                    pend.append((g2, *transpose_group(g2)))
                if k0 > 0:
                    for g2, xh, xl in pend[: len(batch)]:
                        matmul_group(g2, xh, xl)
                    pend = pend[len(batch):]
            for g2, xh, xl in pend:
                matmul_group(g2, xh, xl)

            # ---- back to token-major ----
            lg = stage.tile([128, BLOCK], F32, name=f"lg{b}", tag="lg")
            nc.scalar.copy(lg[:], lp[:])
            npb = nps.tile([128, BLOCK], F32, name=f"np{b}", tag="np")
            for tt in range(TPB):
                nc.tensor.transpose(
                    npb[:, tt * 128 : (tt + 1) * 128],
                    lg[:, tt * 128 : (tt + 1) * 128],
                    eye[:],
                )

            # ---- router math per 128-token tile ----
            prs = rout.tile([128, TPB * E], F32, name=f"prs{b}", tag="prs")
            ixs = rout.tile([128, TPB * TOP_K], U32, name=f"ixs{b}", tag="ixs")
            for tt in range(TPB):
                gate = npb[:, tt * 128 : tt * 128 + E]
                noise = npb[:, tt * 128 + E : tt * 128 + PAIR]
                en = rout.tile([128, E], F32, name=f"en{b}_{tt}", tag="en")
                nc.scalar.activation(en[:], noise, AF.Exp)
                sp = rout.tile([128, E], F32, name=f"sp{b}_{tt}", tag="sp")
                nc.scalar.activation(sp[:], en[:], AF.Ln, bias=1.0)
                gsl = gs[:, (b * TPB + tt) * E : (b * TPB + tt + 1) * E]
                pr = rout.tile([128, E], F32, name=f"pr{b}_{tt}", tag="pr")
                nc.vector.tensor_tensor(pr[:], gsl, sp[:], op=ALU.mult)
                nz = rout.tile([128, E], F32, name=f"nz{b}_{tt}", tag="nz")
                nc.vector.tensor_tensor(nz[:], gate, pr[:], op=ALU.add)
                mx = rout.tile([128, TOP_K], F32, name=f"mx{b}_{tt}", tag="mx")
                nc.vector.max(mx[:], nz[:])
                nc.vector.max_index(
                    ixs[:, tt * TOP_K : (tt + 1) * TOP_K], mx[:], nz[:]
                )
                ez = rout.tile([128, E], F32, name=f"ez{b}_{tt}", tag="ez")
                nc.scalar.activation(ez[:], nz[:], AF.Exp)
                me = rout.tile([128, E], F32, name=f"me{b}_{tt}", tag="me")
                ssum = rout.tile([128, 1], F32, name=f"ss{b}_{tt}", tag="ss")
                nc.vector.scalar_tensor_tensor(
                    me[:], nz[:], mx[:, TOP_K - 1 : TOP_K], ez[:],
                    op0=ALU.is_ge, op1=ALU.mult, accum_out=ssum[:],
                )
                rcp = rout.tile([128, 1], F32, name=f"rc{b}_{tt}", tag="rc")
                nc.vector.reciprocal(rcp[:], ssum[:])
                nc.vector.tensor_scalar(
                    prs[:, tt * E : (tt + 1) * E], me[:], rcp[:, 0:1], None,
                    op0=ALU.mult,
                )

            nc.sync.dma_start(pr_d[:, b * TPB * E : (b + 1) * TPB * E], prs[:])
            nc.sync.dma_start(
                ix_d[:, b * TPB * TOP_K : (b + 1) * TPB * TOP_K], ixs[:]
            )


_NC_CACHE = None


def _get_program():
    global _NC_CACHE
    if _NC_CACHE is None:
        _NC_CACHE = _build_program()
    return _NC_CACHE


def _prep_in_maps(x, gate_w, gate_b, noise_w, noise_b, gauss):
    x = np.asarray(x, dtype=np.float32)
    gauss = np.asarray(gauss, dtype=np.float32)
    gate_w = np.asarray(gate_w, dtype=np.float32)
    noise_w = np.asarray(noise_w, dtype=np.float32)
    gate_b = np.asarray(gate_b, dtype=np.float32)
    noise_b = np.asarray(noise_b, dtype=np.float32)

    w_pair = np.concatenate([gate_w, noise_w], axis=0)            # [128, D]
    # w_sb[p, c*PAIR + e] = w_pair[e, c*128 + p]
    w_sb = np.ascontiguousarray(
        w_pair.T.reshape(CH, 128, PAIR).transpose(1, 0, 2).reshape(128, CH * PAIR)
    )
    wh = w_sb.astype(np.float16)
    wl = (w_sb - wh.astype(np.float32)).astype(np.float16)
    b_pair = np.concatenate([gate_b, noise_b]).astype(np.float32)
    bh = b_pair.astype(np.float16)
    bl = (b_pair - bh.astype(np.float32)).astype(np.float16)
    eye = np.eye(128, dtype=np.float32)

    x2 = x.reshape(N_TOK, DIM)
    g2 = gauss.reshape(N_TOK, E)

    in_maps = []
    for i in range(N_CORES):
        gsl = g2[i * T_CORE : (i + 1) * T_CORE]
        g_sb = np.ascontiguousarray(
            gsl.reshape(NB * TPB, 128, E).transpose(1, 0, 2).reshape(128, NB * TPB * E)
        )
        in_maps.append(
            {
                "x": x2[i * T_CORE : (i + 1) * T_CORE],
                "wh": wh,
                "wl": wl,
                "bh": bh.reshape(1, PAIR),
                "bl": bl.reshape(1, PAIR),
                "g": g_sb,
                "eye": eye,
            }
        )
    return in_maps


def _assemble(results):
    probs = np.empty((N_TOK, E), dtype=np.float32)
    idx = np.empty((N_TOK, TOP_K), dtype=np.int32)
    for i in range(N_CORES):
        r = results[i]
        p = r["probs"].reshape(128, NB * TPB, E).transpose(1, 0, 2).reshape(T_CORE, E)
        probs[i * T_CORE : (i + 1) * T_CORE] = p
        ii = (
            r["idx"]
            .view(np.int32)
            .reshape(128, NB * TPB, TOP_K)
            .transpose(1, 0, 2)
            .reshape(T_CORE, TOP_K)
        )
        idx[i * T_CORE : (i + 1) * T_CORE] = ii
    return probs.reshape(B, S, E), idx.reshape(B, S, TOP_K)


def kernel(x, gate_w, gate_b, noise_w, noise_b, gauss):
    from concourse.bass_utils import run_bass_kernel_spmd

    in_maps = _prep_in_maps(x, gate_w, gate_b, noise_w, noise_b, gauss)
    nc = _get_program()
    res = run_bass_kernel_spmd(nc, in_maps, core_ids=list(range(N_CORES)))
    return _assemble(res.results)


# revision 4
# speedup vs baseline: 1.4132x; 1.0074x over previous
"""NoisyRouter Trainium2 kernel.

Computes, for x:[B,S,D] f32, gate_w/noise_w:[E,D], gate_b/noise_b:[E],
gauss:[B,S,E]:
    logits       = x @ gate_w.T + gate_b
    noise_logits = x @ noise_w.T + noise_b
    noisy        = logits + gauss * softplus(noise_logits)
    top_vals, top_idx = top_k(noisy, 8)
    sparse_logits = softmax(scatter(-inf, top_idx, top_vals))
Returns (sparse_logits:[B,S,E] f32, top_idx:[B,S,8] int32).

Distribution: pure data-parallel over tokens — each of the 8 NeuronCores
gets B*S/8 = 2048 tokens; the small router weights are replicated.

Per-core pipeline (fp32 data, fp16 hi/lo "3x" matmul for speed):
  x rows stream in natural [token, D] layout -> PE transposes 128x128
  tiles into PSUM (x^T) -> ACT copies PSUM->SBUF casting to fp16 (hi)
  while DVE computes lo = fp32(x^T) - hi (fp16). The dual-projection
  matmul runs expert-major with the gate|noise weight pair packed on
  128 partitions: logits^T[pair, tok] accumulates w_hi.T@x_hi +
  w_hi.T@x_lo + w_lo.T@x_hi (fp16 pair arithmetic ~= fp32 precision at
  1 PE cycle/row instead of 4 for fp32). Biases enter as rank-1 K=1
  matmuls. The PE instruction stream is software-pipelined in batches
  (transposes of batch k+1 between the matmuls of batches k and k+1) so
  matmul bursts stay long enough to hold the PE clock at full rate.
  logits^T transposes back to token-major, then per 128-token tile:
  softplus(noise) = Ln(Exp(noise)+1) on ACT, noisy on DVE, DVE
  max8/max_index give the top-8 (sorted) values+indices, and the masked
  softmax uses a fused compare-multiply-accumulate against the 8th max.
"""
import sys

sys.path.insert(0, "/opt/trn_rl_repo")

import numpy as np

import concourse.bass as bass
import concourse.tile as tile
from concourse import mybir
from concourse.vector_clock import ScopedClock

F32 = mybir.dt.float32
F16 = mybir.dt.float16
U32 = mybir.dt.uint32
AF = mybir.ActivationFunctionType
ALU = mybir.AluOpType

B, S, DIM, E, TOP_K = 4, 4096, 4096, 64, 8
N_CORES = 8
N_TOK = B * S
T_CORE = N_TOK // N_CORES          # 2048 tokens per core
PAIR = 2 * E                       # gate|noise packed: 128
CH = DIM // 128                    # 32 contraction chunks
BLOCK = 512                        # tokens per compute block
NB = T_CORE // BLOCK               # blocks per core
TPB = BLOCK // 128                 # 128-token tiles per block: 4
GRP = 2                            # contraction chunks per hi/lo staging group
NG = CH // GRP                     # 16 groups per block
BATCH = 3                          # staging groups per PE pipeline batch


class _TileContextSplitWaits(tile.TileContext):
    """The walrus codegen used here accepts only ONE sync-wait command per
    instruction; Tile can emit several. Split the extras onto same-engine
    NOPs placed immediately before the instruction."""

    def _drain_and_barrier(self, tick_clock, wait_clock):
        nc = self.nc
        drain_inst = nc.sync.drain()
        wait_clock.add_sem_waits(
            drain_inst.ins, ScopedClock({None: tick_clock.global_clock})
        )
        nc.all_engine_barrier()
        assert self.sems is not None
        popped = nc._tile_sem_poison_stack.pop()
        assert popped is self._sem_poison
        nc.clear_and_free_semaphores(list(self.sems.allocated().values()))
        nc.all_engine_barrier()

    def schedule_and_allocate(self):
        ret = super().schedule_and_allocate()
        nc = self.nc
        for bb in nc.bb_map.values():
            insts = bb.bb.instructions
            i = 0
            while i < len(insts):
                inst = insts[i]
                si = inst.sync_info
                if si is not None and si.on_wait and len(si.on_wait) > 1:
                    waits = list(si.on_wait)
                    si.on_wait = [waits[-1]]
                    for k, w in enumerate(waits[:-1]):
                        nop = mybir.InstNoOp(
                            name=f"{inst.name}-w{k}",
                            engine=inst.engine,
                            ins=[],
                            outs=[],
                            sync_info=mybir.SyncInfo(on_wait=[w], on_update=[]),
                        )
                        nc.register_instruction(nop, overwrite=True)
                        insts.insert(i, nop)
                        i += 1
                i += 1
        return ret


def _build_program():
    nc = bass.Bass(
        "TRN2", target_bir_lowering=False, debug=False, num_devices=N_CORES
    )
    x_d = nc.dram_tensor("x", [T_CORE, DIM], F32, kind="ExternalInput").ap()
    # weights pre-arranged on host to the SBUF-native fp16 hi/lo layout:
    # w?_sb[p, c*PAIR + e] = hi/lo(w_pair[e, c*128 + p])
    wh_d = nc.dram_tensor("wh", [128, CH * PAIR], F16, kind="ExternalInput").ap()
    wl_d = nc.dram_tensor("wl", [128, CH * PAIR], F16, kind="ExternalInput").ap()
    bh_d = nc.dram_tensor("bh", [1, PAIR], F16, kind="ExternalInput").ap()
    bl_d = nc.dram_tensor("bl", [1, PAIR], F16, kind="ExternalInput").ap()
    # gauss pre-arranged: g_sb[p, ((b*TPB+tt)*E + e)] = gauss[b*BLOCK+tt*128+p, e]
    g_d = nc.dram_tensor("g", [128, NB * TPB * E], F32, kind="ExternalInput").ap()
    eye_d = nc.dram_tensor("eye", [128, 128], F32, kind="ExternalInput").ap()
    # outputs in the same partition-major layout; host undoes it
    pr_d = nc.dram_tensor("probs", [128, NB * TPB * E], F32, kind="ExternalOutput").ap()
    ix_d = nc.dram_tensor(
        "idx", [128, NB * TPB * TOP_K], U32, kind="ExternalOutput"
    ).ap()

    with _TileContextSplitWaits(nc) as tc:
        _emit(nc, tc, x_d, wh_d, wl_d, bh_d, bl_d, g_d, eye_d, pr_d, ix_d)
    return nc


def _emit(nc, tc, x_d, wh_d, wl_d, bh_d, bl_d, g_d, eye_d, pr_d, ix_d):
    from contextlib import ExitStack

    ctx = ExitStack()
    with ctx:
        const = ctx.enter_context(tc.tile_pool(name="const", bufs=1))
        xpool = ctx.enter_context(tc.tile_pool(name="xpool", bufs=8))
        stage = ctx.enter_context(tc.tile_pool(name="stage", bufs=3))
        rout = ctx.enter_context(tc.tile_pool(name="rout", bufs=3))
        xtps = ctx.enter_context(tc.tile_pool(name="xtps", bufs=BATCH, space="PSUM"))
        lps = ctx.enter_context(tc.tile_pool(name="lps", bufs=1, space="PSUM"))
        nps = ctx.enter_context(tc.tile_pool(name="nps", bufs=1, space="PSUM"))

        # eye first (transposes need it), then block-0 x, then weights,
        # then gauss — DMA queue order controls when compute can start.
        eye = const.tile([128, 128], F32)
        nc.sync.dma_start(eye[:], eye_d[:])

        xs_all = []
        def load_block(b, n_pieces=1):
            # n_pieces>1 splits each sub-tile row-block into column pieces,
            # DMA'd in column-major piece order so the first contraction
            # chunks of all four sub-tiles land first (fast compute start).
            xs = [
                xpool.tile([128, DIM], F32, name=f"x_{b}_{tt}", tag="x")
                for tt in range(TPB)
            ]
            w = DIM // n_pieces
            for q in range(n_pieces):
                for tt in range(TPB):
                    nc.sync.dma_start(
                        xs[tt][:, q * w : (q + 1) * w],
                        x_d[
                            b * BLOCK + tt * 128 : b * BLOCK + (tt + 1) * 128,
                            q * w : (q + 1) * w,
                        ],
                    )
            return xs

        # block 0: first column pieces of all four sub-tiles, then the
        # weights (so matmuls can start early), then the rest of the block
        xs0 = [
            xpool.tile([128, DIM], F32, name=f"x_0_{tt}", tag="x")
            for tt in range(TPB)
        ]
        NP0, W0 = 8, DIM // 8
        for tt in range(TPB):
            nc.sync.dma_start(
                xs0[tt][:, 0:W0], x_d[tt * 128 : (tt + 1) * 128, 0:W0]
            )
        wh = const.tile([128, CH * PAIR], F16)
        nc.sync.dma_start(wh[:], wh_d[:])
        wl = const.tile([128, CH * PAIR], F16)
        nc.sync.dma_start(wl[:], wl_d[:])
        for q in range(1, NP0):
            for tt in range(TPB):
                nc.sync.dma_start(
                    xs0[tt][:, q * W0 : (q + 1) * W0],
                    x_d[tt * 128 : (tt + 1) * 128, q * W0 : (q + 1) * W0],
                )
        xs_all.append(xs0)
        bh_s = const.tile([1, PAIR], F16)
        nc.sync.dma_start(bh_s[:], bh_d[:])
        bl_s = const.tile([1, PAIR], F16)
        nc.sync.dma_start(bl_s[:], bl_d[:])
        ones = const.tile([1, BLOCK], F16)
        nc.vector.memset(ones[:], 1.0)
        gs = const.tile([128, NB * TPB * E], F32)
        nc.sync.dma_start(gs[:], g_d[:])

        for b in range(NB):
            if b + 1 < NB:
                xs_all.append(load_block(b + 1))
            xs = xs_all[b]

            lp = lps.tile([128, BLOCK], F32, name=f"lp{b}", tag="lp")
            nc.tensor.matmul(lp[:], bh_s[0:1, :], ones[0:1, :], start=True, stop=False)
            nc.tensor.matmul(lp[:], bl_s[0:1, :], ones[0:1, :], start=False, stop=False)

            # ---- software-pipelined PE stream over staging groups ----
            # batch k: transposes emitted before batch k-1's matmuls so the
            # PE alternates [T-batch][M-batch] with no bubbles and long
            # matmul bursts.
            def transpose_group(g2):
                xt = xtps.tile(
                    [128, GRP * BLOCK], F32, name=f"xt{b}_{g2}", tag="xt"
                )
                for j in range(GRP):
                    c = g2 * GRP + j
                    for tt in range(TPB):
                        nc.tensor.transpose(
                            xt[:, j * BLOCK + tt * 128 : j * BLOCK + (tt + 1) * 128],
                            xs[tt][:, c * 128 : (c + 1) * 128],
                            eye[:],
                        )
                xh = stage.tile([128, GRP * BLOCK], F16, name=f"xh{b}_{g2}", tag="xh")
                nc.scalar.copy(xh[:], xt[:])
                xl = stage.tile([128, GRP * BLOCK], F16, name=f"xl{b}_{g2}", tag="xl")
                nc.vector.scalar_tensor_tensor(
                    xl[:], xt[:], 0.0, xh[:], op0=ALU.add, op1=ALU.subtract
                )
                return xh, xl

            def matmul_group(g2, xh, xl):
                for j in range(GRP):
                    c = g2 * GRP + j
                    wsl = slice(c * PAIR, (c + 1) * PAIR)
                    xsl = slice(j * BLOCK, (j + 1) * BLOCK)
                    last = c == CH - 1
                    nc.tensor.matmul(
                        lp[:], wh[:, wsl], xh[:, xsl], start=False, stop=False
                    )
                    nc.tensor.matmul(
                        lp[:], wh[:, wsl], xl[:, xsl], start=False, stop=False
                    )
                    nc.tensor.matmul(
                        lp[:], wl[:, wsl], xh[:, xsl], start=False, stop=last
                    )

            pend = []  # [(g2, xh, xl)] transposed but not yet matmul'ed
            for k0 in range(0, NG, BATCH):
                batch = list(range(k0, min(k0 + BATCH, NG)))
                for g2 in batch:
                    pend.append((g2, *transpose_group(g2)))
                if k0 > 0:
                    for g2, xh, xl in pend[: len(batch)]:
                        matmul_group(g2, xh, xl)
                    pend = pend[len(batch):]
            for g2, xh, xl in pend:
                matmul_group(g2, xh, xl)

            # ---- back to token-major ----
            lg = stage.tile([128, BLOCK], F32, name=f"lg{b}", tag="lg")
            nc.scalar.copy(lg[:], lp[:])
            npb = nps.tile([128, BLOCK], F32, name=f"np{b}", tag="np")
            for tt in range(TPB):
                nc.tensor.transpose(
                    npb[:, tt * 128 : (tt + 1) * 128],
                    lg[:, tt * 128 : (tt + 1) * 128],
                    eye[:],
                )

            # ---- router math per 128-token tile ----
            prs = rout.tile([128, TPB * E], F32, name=f"prs{b}", tag="prs")
            ixs = rout.tile([128, TPB * TOP_K], U32, name=f"ixs{b}", tag="ixs")
            for tt in range(TPB):
                gate = npb[:, tt * 128 : tt * 128 + E]
                noise = npb[:, tt * 128 + E : tt * 128 + PAIR]
                en = rout.tile([128, E], F32, name=f"en{b}_{tt}", tag="en")
                nc.scalar.activation(en[:], noise, AF.Exp)
                sp = rout.tile([128, E], F32, name=f"sp{b}_{tt}", tag="sp")
                nc.scalar.activation(sp[:], en[:], AF.Ln, bias=1.0)
                gsl = gs[:, (b * TPB + tt) * E : (b * TPB + tt + 1) * E]
                pr = rout.tile([128, E], F32, name=f"pr{b}_{tt}", tag="pr")
                nc.vector.tensor_tensor(pr[:], gsl, sp[:], op=ALU.mult)
                nz = rout.tile([128, E], F32, name=f"nz{b}_{tt}", tag="nz")
                nc.vector.tensor_tensor(nz[:], gate, pr[:], op=ALU.add)
                mx = rout.tile([128, TOP_K], F32, name=f"mx{b}_{tt}", tag="mx")
                nc.vector.max(mx[:], nz[:])
                nc.vector.max_index(
                    ixs[:, tt * TOP_K : (tt + 1) * TOP_K], mx[:], nz[:]
                )
                ez = rout.tile([128, E], F32, name=f"ez{b}_{tt}", tag="ez")
                nc.scalar.activation(ez[:], nz[:], AF.Exp)
                me = rout.tile([128, E], F32, name=f"me{b}_{tt}", tag="me")
                ssum = rout.tile([128, 1], F32, name=f"ss{b}_{tt}", tag="ss")
                nc.vector.scalar_tensor_tensor(
                    me[:], nz[:], mx[:, TOP_K - 1 : TOP_K], ez[:],
                    op0=ALU.is_ge, op1=ALU.mult, accum_out=ssum[:],
                )
                rcp = rout.tile([128, 1], F32, name=f"rc{b}_{tt}", tag="rc")
                nc.vector.reciprocal(rcp[:], ssum[:])
                nc.vector.tensor_scalar(
                    prs[:, tt * E : (tt + 1) * E], me[:], rcp[:, 0:1], None,
                    op0=ALU.mult,
                )

            nc.sync.dma_start(pr_d[:, b * TPB * E : (b + 1) * TPB * E], prs[:])
            nc.sync.dma_start(
                ix_d[:, b * TPB * TOP_K : (b + 1) * TPB * TOP_K], ixs[:]
            )


_NC_CACHE = None


def _get_program():
    global _NC_CACHE
    if _NC_CACHE is None:
        _NC_CACHE = _build_program()
    return _NC_CACHE


def _prep_in_maps(x, gate_w, gate_b, noise_w, noise_b, gauss):
    x = np.asarray(x, dtype=np.float32)
    gauss = np.asarray(gauss, dtype=np.float32)
    gate_w = np.asarray(gate_w, dtype=np.float32)
    noise_w = np.asarray(noise_w, dtype=np.float32)
    gate_b = np.asarray(gate_b, dtype=np.float32)
    noise_b = np.asarray(noise_b, dtype=np.float32)

    w_pair = np.concatenate([gate_w, noise_w], axis=0)            # [128, D]
    # w_sb[p, c*PAIR + e] = w_pair[e, c*128 + p]
    w_sb = np.ascontiguousarray(
        w_pair.T.reshape(CH, 128, PAIR).transpose(1, 0, 2).reshape(128, CH * PAIR)
    )
    wh = w_sb.astype(np.float16)
    wl = (w_sb - wh.astype(np.float32)).astype(np.float16)
    b_pair = np.concatenate([gate_b, noise_b]).astype(np.float32)
    bh = b_pair.astype(np.float16)
    bl = (b_pair - bh.astype(np.float32)).astype(np.float16)
    eye = np.eye(128, dtype=np.float32)

    x2 = x.reshape(N_TOK, DIM)
    g2 = gauss.reshape(N_TOK, E)

    in_maps = []
    for i in range(N_CORES):
        gsl = g2[i * T_CORE : (i + 1) * T_CORE]
        g_sb = np.ascontiguousarray(
            gsl.reshape(NB * TPB, 128, E).transpose(1, 0, 2).reshape(128, NB * TPB * E)
        )
        in_maps.append(
            {
                "x": x2[i * T_CORE : (i + 1) * T_CORE],
                "wh": wh,
                "wl": wl,
                "bh": bh.reshape(1, PAIR),
                "bl": bl.reshape(1, PAIR),
                "g": g_sb,
                "eye": eye,
            }
        )
    return in_maps


def _assemble(results):
    probs = np.empty((N_TOK, E), dtype=np.float32)
    idx = np.empty((N_TOK, TOP_K), dtype=np.int32)
    for i in range(N_CORES):
        r = results[i]
        p = r["probs"].reshape(128, NB * TPB, E).transpose(1, 0, 2).reshape(T_CORE, E)
        probs[i * T_CORE : (i + 1) * T_CORE] = p
        ii = (
            r["idx"]
            .view(np.int32)
            .reshape(128, NB * TPB, TOP_K)
            .transpose(1, 0, 2)
            .reshape(T_CORE, TOP_K)
        )
        idx[i * T_CORE : (i + 1) * T_CORE] = ii
    return probs.reshape(B, S, E), idx.reshape(B, S, TOP_K)


def kernel(x, gate_w, gate_b, noise_w, noise_b, gauss):
    from concourse.bass_utils import run_bass_kernel_spmd

    in_maps = _prep_in_maps(x, gate_w, gate_b, noise_w, noise_b, gauss)
    nc = _get_program()
    res = run_bass_kernel_spmd(nc, in_maps, core_ids=list(range(N_CORES)))
    return _assemble(res.results)
